# revision 8
# baseline (speedup 1.0000x reference)
"""Trainium2 Bass kernel for nn_ExLoss: tensor-parallel classifier over a
memory bank.

reference computes:
    tsims   = inputs @ V.T                    # [B, C]
    outputs = tsims * T                       # T = 1.0
    bu_loss = mean cross-entropy(outputs, targets)
    loss    = W_MS * ms_loss + W_BU * bu_loss # W_MS = 0.0, W_BU = 1.0
    return (loss, outputs)

Sharding: V (and thus the [B, C] logits) is split over classes across the 8
NeuronCores.  Each core computes its [B, C/8] logit block, streams it to HBM,
and produces per-row partial sum(exp(logit)) for the softmax denominator.
Since inputs and V rows are unit-norm, logits are in [-1, 1] and the
logsumexp needs no max subtraction.  The tiny cross-core combine (8 partial
sumexp vectors -> logsumexp, target-logit gather, mean) runs on the host.
ms_loss has weight 0.0 and is skipped entirely.
"""

import numpy as np

B, D, C = 4096, 256, 32000
N_CORES = 8
CS = C // N_CORES          # 4000 classes per core
P = 128                    # partitions
M_TILES = B // P           # 32 row blocks
K_TILES = D // P           # 2 contraction tiles
CHUNK = 500                # psum chunk width (<=512 fp32 = one psum bank)
N_CHUNKS = CS // CHUNK     # 8

# "f32r": fp32 data, PE fast mode (1 cycle/row at N>=256)
# "f32" : exact fp32 matmul (4 cycles/row)
# "bf16": inputs cast to bf16 (1 cycle/row)
MM_DTYPE = "f32r"

_CACHE = {}


def _build_nc(mm_dtype):
    from contextlib import ExitStack

    import concourse.mybir as mybir
    import concourse.tile as tile
    from concourse import bacc
    from concourse.bass import ts

    f32 = mybir.dt.float32
    if mm_dtype == "bf16":
        in_dt = mybir.dt.bfloat16
    elif mm_dtype == "f32r":
        # fp32 bytes in memory; PE reads them in fast (reduced-precision)
        # fp32 mode at bf16-rate.  Declaring the DRAM/SBUF tensors as
        # float32r satisfies the BIR verifier's rounding rule.
        in_dt = mybir.dt.float32r
    else:
        in_dt = f32

    nc = bacc.Bacc("TRN2", target_bir_lowering=False, debug=False,
                   num_devices=N_CORES)
    xt_ap = nc.dram_tensor("xt", [D, B], in_dt, kind="ExternalInput").ap()
    vt_ap = nc.dram_tensor("vt", [D, CS], in_dt, kind="ExternalInput").ap()
    out_ap = nc.dram_tensor("out", [B, CS], f32, kind="ExternalOutput").ap()
    se_ap = nc.dram_tensor("sumexp", [P, M_TILES], f32,
                           kind="ExternalOutput").ap()

    GRP = 4 * CHUNK            # 2000 cols = 4 psum banks per group
    N_GRPS = CS // GRP         # 2 groups per row block

    with tile.TileContext(nc) as tc, ExitStack() as ctx:
        inp = ctx.enter_context(tc.tile_pool(name="inp", bufs=1))
        sp = ctx.enter_context(tc.tile_pool(name="s", bufs=3))
        pp = ctx.enter_context(tc.tile_pool(name="ps", bufs=2, space="PSUM"))
        ep = ctx.enter_context(tc.tile_pool(name="es", bufs=2))
        fp = ctx.enter_context(tc.tile_pool(name="fin", bufs=1))

        # Chunked input loads so the first matmuls only wait on the first
        # slices instead of the whole 8 MB.
        xts, vts = [], []
        for k in range(K_TILES):
            vts.append(inp.tile([P, CS], in_dt, tag=f"vt{k}",
                                name=f"vt_t{k}"))
            xts.append(inp.tile([P, B], in_dt, tag=f"xt{k}",
                                name=f"xt_t{k}"))
        VCH = 1000
        for j in range(CS // VCH):
            for k in range(K_TILES):
                nc.sync.dma_start(vts[k][:, ts(j, VCH)],
                                  vt_ap[ts(k, P), ts(j, VCH)])
        XCH = 1024
        for j in range(B // XCH):
            for k in range(K_TILES):
                nc.sync.dma_start(xts[k][:, ts(j, XCH)],
                                  xt_ap[ts(k, P), ts(j, XCH)])

        bias0 = fp.tile([P, 1], f32, tag="bias0")
        nc.any.memset(bias0[:], 0.0)
        se_t = fp.tile([P, M_TILES], f32, tag="se")

        for m in range(M_TILES):
            s_t = sp.tile([P, CS], f32, tag="s")
            for g in range(N_GRPS):
                # 4 chunks per psum tile; 512-col (one bank) stride so each
                # matmul output stays inside a single psum bank.
                ps = pp.tile([P, 4, 512], f32, tag="ps")
                for j in range(4):
                    ch = g * 4 + j
                    for k in range(K_TILES):
                        nc.tensor.matmul(ps[:, j, 0:CHUNK],
                                         xts[k][:, ts(m, P)],
                                         vts[k][:, ts(ch, CHUNK)],
                                         start=(k == 0),
                                         stop=(k == K_TILES - 1))
                dst = s_t[:, ts(g, GRP)].rearrange("p (j c) -> p j c",
                                                   c=CHUNK)
                nc.vector.tensor_copy(dst, ps[:, :, 0:CHUNK])
                # stream each half-row-block out as soon as it is assembled
                nc.sync.dma_start(out_ap[ts(m, P), ts(g, GRP)],
                                  s_t[:, ts(g, GRP)])
            es = ep.tile([P, CS], f32, tag="es")
            nc.scalar.activation(es[:], s_t[:],
                                 mybir.ActivationFunctionType.Exp,
                                 bias=bias0[:],
                                 accum_out=se_t[:, ts(m, 1)])
        nc.sync.dma_start(se_ap[:], se_t[:])

    nc.compile()
    return nc


def _get_nc(mm_dtype=MM_DTYPE):
    if mm_dtype not in _CACHE:
        _CACHE[mm_dtype] = _build_nc(mm_dtype)
    return _CACHE[mm_dtype]


def _run(inputs, V, mm_dtype=MM_DTYPE, trace=False):
    from concourse.bass_utils import run_bass_kernel_spmd

    nc = _get_nc(mm_dtype)
    if mm_dtype == "bf16":
        import ml_dtypes
        host_dt = ml_dtypes.bfloat16
    else:
        host_dt = np.float32
    XT = np.ascontiguousarray(inputs.T.astype(host_dt))
    in_maps = []
    for c in range(N_CORES):
        VT = np.ascontiguousarray(V[c * CS:(c + 1) * CS].T.astype(host_dt))
        in_maps.append({"xt": XT, "vt": VT})
    res = run_bass_kernel_spmd(nc, in_maps, core_ids=list(range(N_CORES)),
                               trace=trace)
    return res


def kernel(inputs, targets, indexs=None, label_to_pairs=None,
           all_label_to_clusterid=None, V=None, _trace=False, _res_out=None):
    inputs = np.asarray(inputs, dtype=np.float32)
    V = np.asarray(V, dtype=np.float32)
    targets = np.asarray(targets).astype(np.int64)

    res = _run(inputs, V, trace=_trace)
    if _res_out is not None:
        _res_out.append(res)

    outputs = np.concatenate([r["out"] for r in res.results], axis=1)
    # sumexp tile layout: [partition p, m] holds row m*128 + p
    sumexp = np.zeros(B, dtype=np.float64)
    for r in res.results:
        sumexp += r["sumexp"].T.reshape(-1).astype(np.float64)
    lse = np.log(sumexp)
    t_logit = outputs[np.arange(B), targets].astype(np.float64)
    loss = np.float32((lse - t_logit).mean())
    return (loss, outputs)


# revision 9
# speedup vs baseline: 1.1832x; 1.1832x over previous
"""Trainium2 Bass kernel for nn_ExLoss: tensor-parallel classifier over a
memory bank.

reference computes:
    tsims   = inputs @ V.T                    # [B, C]
    outputs = tsims * T                       # T = 1.0
    bu_loss = mean cross-entropy(outputs, targets)
    loss    = W_MS * ms_loss + W_BU * bu_loss # W_MS = 0.0, W_BU = 1.0
    return (loss, outputs)

Sharding: V (and thus the [B, C] logits) is split over classes across the 8
NeuronCores.  Each core computes its [B, C/8] logit block, streams it to HBM,
and produces per-row partial sum(exp(logit)) for the softmax denominator.
Since inputs and V rows are unit-norm, logits are in [-1, 1] and the
logsumexp needs no max subtraction.  The tiny cross-core combine (8 partial
sumexp vectors -> logsumexp, target-logit gather, mean) runs on the host.
ms_loss has weight 0.0 and is skipped entirely.
"""

import numpy as np

B, D, C = 4096, 256, 32000
N_CORES = 8
CS = C // N_CORES          # 4000 classes per core
P = 128                    # partitions
M_TILES = B // P           # 32 row blocks
K_TILES = D // P           # 2 contraction tiles
CHUNK = 500                # psum chunk width (<=512 fp32 = one psum bank)
N_CHUNKS = CS // CHUNK     # 8

# "f32r": fp32 data, PE fast mode (1 cycle/row at N>=256)
# "f32" : exact fp32 matmul (4 cycles/row)
# "bf16": inputs cast to bf16 (1 cycle/row)
MM_DTYPE = "f32r"

_CACHE = {}


def _build_nc(mm_dtype):
    from contextlib import ExitStack

    import concourse.mybir as mybir
    import concourse.tile as tile
    from concourse import bacc
    from concourse.bass import ts

    f32 = mybir.dt.float32
    if mm_dtype == "bf16":
        in_dt = mybir.dt.bfloat16
    elif mm_dtype == "f32r":
        # fp32 bytes in memory; PE reads them in fast (reduced-precision)
        # fp32 mode at bf16-rate.  Declaring the DRAM/SBUF tensors as
        # float32r satisfies the BIR verifier's rounding rule.
        in_dt = mybir.dt.float32r
    else:
        in_dt = f32

    nc = bacc.Bacc("TRN2", target_bir_lowering=False, debug=False,
                   num_devices=N_CORES)
    xt_ap = nc.dram_tensor("xt", [D, B], in_dt, kind="ExternalInput").ap()
    vt_ap = nc.dram_tensor("vt", [D, CS], in_dt, kind="ExternalInput").ap()
    out_ap = nc.dram_tensor("out", [B, CS], f32, kind="ExternalOutput").ap()
    se_ap = nc.dram_tensor("sumexp", [P, M_TILES], f32,
                           kind="ExternalOutput").ap()

    GRP = 4 * CHUNK            # 2000 cols = 4 psum banks per group
    N_GRPS = CS // GRP         # 2 groups per row block

    with tile.TileContext(nc) as tc, ExitStack() as ctx:
        inp = ctx.enter_context(tc.tile_pool(name="inp", bufs=1))
        sp = ctx.enter_context(tc.tile_pool(name="s", bufs=3))
        pp = ctx.enter_context(tc.tile_pool(name="ps", bufs=2, space="PSUM"))
        ep = ctx.enter_context(tc.tile_pool(name="es", bufs=2))
        fp = ctx.enter_context(tc.tile_pool(name="fin", bufs=1))

        # Chunked input loads so the first matmuls only wait on the first
        # slices instead of the whole 8 MB.
        xts, vts = [], []
        for k in range(K_TILES):
            vts.append(inp.tile([P, CS], in_dt, tag=f"vt{k}",
                                name=f"vt_t{k}"))
            xts.append(inp.tile([P, B], in_dt, tag=f"xt{k}",
                                name=f"xt_t{k}"))
        VCH = 1000
        for j in range(CS // VCH):
            for k in range(K_TILES):
                nc.sync.dma_start(vts[k][:, ts(j, VCH)],
                                  vt_ap[ts(k, P), ts(j, VCH)])
        XCH = 1024
        for j in range(B // XCH):
            for k in range(K_TILES):
                nc.sync.dma_start(xts[k][:, ts(j, XCH)],
                                  xt_ap[ts(k, P), ts(j, XCH)])

        bias0 = fp.tile([P, 1], f32, tag="bias0")
        nc.any.memset(bias0[:], 0.0)
        se_t = fp.tile([P, M_TILES], f32, tag="se")

        for m in range(M_TILES):
            s_t = sp.tile([P, CS], f32, tag="s")
            for g in range(N_GRPS):
                # 4 chunks per psum tile; 512-col (one bank) stride so each
                # matmul output stays inside a single psum bank.
                ps = pp.tile([P, 4, 512], f32, tag="ps")
                for j in range(4):
                    ch = g * 4 + j
                    for k in range(K_TILES):
                        nc.tensor.matmul(ps[:, j, 0:CHUNK],
                                         xts[k][:, ts(m, P)],
                                         vts[k][:, ts(ch, CHUNK)],
                                         start=(k == 0),
                                         stop=(k == K_TILES - 1))
                dst = s_t[:, ts(g, GRP)].rearrange("p (j c) -> p j c",
                                                   c=CHUNK)
                nc.vector.tensor_copy(dst, ps[:, :, 0:CHUNK])
            es = ep.tile([P, CS], f32, tag="es")
            nc.scalar.activation(es[:], s_t[:],
                                 mybir.ActivationFunctionType.Exp,
                                 bias=bias0[:],
                                 accum_out=se_t[:, ts(m, 1)])
            nc.sync.dma_start(out_ap[ts(m, P), :], s_t[:])
        nc.sync.dma_start(se_ap[:], se_t[:])

    nc.compile()
    return nc


def _get_nc(mm_dtype=MM_DTYPE):
    if mm_dtype not in _CACHE:
        _CACHE[mm_dtype] = _build_nc(mm_dtype)
    return _CACHE[mm_dtype]


def _run(inputs, V, mm_dtype=MM_DTYPE, trace=False):
    from concourse.bass_utils import run_bass_kernel_spmd

    nc = _get_nc(mm_dtype)
    if mm_dtype == "bf16":
        import ml_dtypes
        host_dt = ml_dtypes.bfloat16
    else:
        host_dt = np.float32
    XT = np.ascontiguousarray(inputs.T.astype(host_dt))
    in_maps = []
    for c in range(N_CORES):
        VT = np.ascontiguousarray(V[c * CS:(c + 1) * CS].T.astype(host_dt))
        in_maps.append({"xt": XT, "vt": VT})
    res = run_bass_kernel_spmd(nc, in_maps, core_ids=list(range(N_CORES)),
                               trace=trace)
    return res


def kernel(inputs, targets, indexs=None, label_to_pairs=None,
           all_label_to_clusterid=None, V=None, _trace=False, _res_out=None):
    inputs = np.asarray(inputs, dtype=np.float32)
    V = np.asarray(V, dtype=np.float32)
    targets = np.asarray(targets).astype(np.int64)

    res = _run(inputs, V, trace=_trace)
    if _res_out is not None:
        _res_out.append(res)

    outputs = np.concatenate([r["out"] for r in res.results], axis=1)
    # sumexp tile layout: [partition p, m] holds row m*128 + p
    sumexp = np.zeros(B, dtype=np.float64)
    for r in res.results:
        sumexp += r["sumexp"].T.reshape(-1).astype(np.float64)
    lse = np.log(sumexp)
    t_logit = outputs[np.arange(B), targets].astype(np.float64)
    loss = np.float32((lse - t_logit).mean())
    return (loss, outputs)
